# revision 1
# baseline (speedup 1.0000x reference)
"""Trainium2 Bass kernel for nn_Blur: depthwise 4x4 FIR conv, pad=2. v6.

v6 = v3 (302us) + CB=16: 16 channels per DMA batch doubles the bytes per
DMA descriptor row (8384B loads / 8224B stores), halving descriptor and
issue counts to unclamp the per-ring dispatch rate. Matmuls run in two
8-channel sub-batches per tile (8 psum banks). Tail FMAs emit one per
tile iteration (32 tiles, 32 FMAs). Copies split 6 vector / 10 scalar.
Stores stay on the gpsimd SWDGE ring (v4/v5 showed scalar-ring stores +
overloaded vector regress).
"""

import numpy as np

_C, _H, _W = 256, 256, 256
_HO, _WO = 257, 257
_NCORES = 8
_TILES = [(0, 125, 0, 126), (125, 125, 123, 128)]
_XW = 262  # per-channel padded width: 2 zero | 256 data | 4 zero
_NMM = 258
_CB = 16  # channels per DMA batch; 2 psum sub-batches of 8
_NTAIL = 7
_FMA_START = 0  # 32 iterations, 32 tail FMAs: one per iteration


def _build_bands(kern):
    wf = np.ascontiguousarray(np.asarray(kern, np.float32)[::-1, ::-1])
    bands = np.zeros((128, 2, 4, 125), np.float32)
    for v, (hp0, Mv, hlo, Kv) in enumerate(_TILES):
        for j in range(4):
            for hr in range(Kv):
                h = hlo + hr
                for mr in range(Mv):
                    i = h - (hp0 + mr) + 2
                    if 0 <= i < 4:
                        bands[hr, v, j, mr] = wf[i, j]
    return bands.astype(np.float16), wf


_NC_CACHE = {}


def _build_nc():
    if "nc" in _NC_CACHE:
        return _NC_CACHE["nc"]
    import concourse.bacc as bacc
    import concourse.mybir as mybir
    import concourse.tile as tile

    f16 = mybir.dt.float16
    f32 = mybir.dt.float32

    nc = bacc.Bacc()
    x_d = nc.declare_dram_parameter("x", [_H, _C * _XW], f16, isOutput=False)
    b_d = nc.declare_dram_parameter("bands", [128, 2, 4, 125], f16, isOutput=False)
    s_d = nc.declare_dram_parameter("strip", [128, 2, 10, 260], f32, isOutput=False)
    w_d = nc.declare_dram_parameter("wfbc", [128, 16], f32, isOutput=False)
    o_d = nc.declare_dram_parameter("out", [_HO, _C * _WO], f16, isOutput=True)
    t_d = nc.declare_dram_parameter("tail", [128, 2, _NTAIL, _WO], f16, isOutput=True)

    NBX = 4
    NBO = 4
    NBP = 8
    XTW = _CB * _XW  # 4192
    OSW = _CB * _WO  # 4112
    TAPS = [(i, j) for i in range(4) for j in range(4)]
    with tile.TileContext(nc) as tc:
        with (
            tc.tile_pool(name="sb", bufs=1) as pool,
            tc.tile_pool(name="ps", bufs=1, space="PSUM") as pp,
        ):
            band_sb = pool.tile([128, 2, 4, 125], f16, tag="bands")
            nc.sync.dma_start(out=band_sb[:], in_=b_d[:])
            strip_sb = pool.tile([128, 2, 10, 260], f32, tag="strip")
            nc.sync.dma_start(out=strip_sb[:], in_=s_d[:])
            wf_sb = pool.tile([128, 16], f32, tag="wf")
            nc.sync.dma_start(out=wf_sb[:], in_=w_d[:])

            xts = [
                pool.tile([128, XTW], f16, tag=f"xt{i}", name=f"xt{i}")
                for i in range(NBX)
            ]
            oss = [
                pool.tile([128, OSW], f16, tag=f"os{i}", name=f"os{i}")
                for i in range(NBO)
            ]
            pss = [
                pp.tile([128, 512], f32, tag=f"ps{i}", name=f"ps{i}")
                for i in range(NBP)
            ]
            accA = pool.tile([128, 2, _NTAIL, _WO], f32, tag="accA")
            accB = pool.tile([128, 2, _NTAIL, _WO], f32, tag="accB")
            acc16 = pool.tile([128, 2, _NTAIL, _WO], f16, tag="acc16")

            def emit_tail_fma(k):
                g, t = k % 2, k // 2
                i, j = TAPS[t]
                src = strip_sb[:, g, i : i + _NTAIL, j : j + _WO]
                sc = wf_sb[:, t : t + 1]
                if t == 0:
                    nc.vector.tensor_scalar_mul(accA[:, g], src, sc)
                else:
                    dst, prev = (accA, accB) if t % 2 == 0 else (accB, accA)
                    nc.vector.scalar_tensor_tensor(
                        out=dst[:, g],
                        in0=src,
                        scalar=sc,
                        in1=prev[:, g],
                        op0=mybir.AluOpType.mult,
                        op1=mybir.AluOpType.add,
                    )
                if t == 15:
                    nc.vector.tensor_copy(acc16[:, g], accB[:, g])
                    if g == 1:
                        nc.gpsimd.dma_start(out=t_d[:], in_=acc16[:])

            it = 0
            for c0 in range(0, _C, _CB):
                for v, (hp0, Mv, hlo, Kv) in enumerate(_TILES):
                    xt = xts[it % NBX]
                    osb = oss[it % NBO]
                    k = it - _FMA_START
                    if 0 <= k < 32:
                        emit_tail_fma(k)
                    nc.sync.dma_start(
                        out=xt[0:Kv, 0:XTW],
                        in_=x_d[hlo : hlo + Kv, c0 * _XW : c0 * _XW + XTW],
                    )
                    for half in range(2):
                        for j in range(4):
                            for c8 in range(8):
                                cc = half * 8 + c8
                                nc.tensor.matmul(
                                    pss[c8][0:Mv, 0:_NMM],
                                    band_sb[0:Kv, v, j, 0:Mv],
                                    xt[0:Kv, cc * _XW + j : cc * _XW + j + _NMM],
                                    start=(j == 0),
                                    stop=(j == 3),
                                )
                        for c8 in range(8):
                            cc = half * 8 + c8
                            ps = pss[c8]
                            if c8 < 3:
                                nc.vector.tensor_copy(
                                    osb[0:Mv, cc * _WO : cc * _WO + _WO],
                                    ps[0:Mv, 0:_WO],
                                )
                            else:
                                nc.scalar.copy(
                                    osb[0:Mv, cc * _WO : cc * _WO + _WO],
                                    ps[0:Mv, 0:_WO],
                                )
                    nc.gpsimd.dma_start(
                        out=o_d[hp0 : hp0 + Mv, c0 * _WO : c0 * _WO + OSW],
                        in_=osb[0:Mv, 0:OSW],
                    )
                    it += 1
    nc.finalize()
    _NC_CACHE["nc"] = nc
    return nc


def _prep_core_inputs(x, bands16, wfbc, b):
    xb = x[b]  # [C, H, W] f32
    xT = np.zeros((_H, _C, _XW), np.float16)
    xT[:, :, 2:258] = xb.transpose(1, 0, 2).astype(np.float16, order="C")
    strip = np.zeros((128, 2, 10, 260), np.float32)
    bot = xb[:, 248:256, :]
    strip[:, 0, 0:8, 2:258] = bot[0:128]
    strip[:, 1, 0:8, 2:258] = bot[128:256]
    return {
        "x": xT.reshape(_H, _C * _XW),
        "bands": bands16,
        "strip": strip,
        "wfbc": wfbc,
    }


def _run(x, kern, trace=False):
    from concourse.bass_utils import run_bass_kernel_spmd

    x = np.asarray(x, dtype=np.float32)
    bands16, wf = _build_bands(kern)
    wfbc = np.ascontiguousarray(
        np.broadcast_to(wf.reshape(1, 16), (128, 16)).astype(np.float32)
    )
    nc = _build_nc()
    in_maps = [_prep_core_inputs(x, bands16, wfbc, b) for b in range(_NCORES)]
    res = run_bass_kernel_spmd(nc, in_maps, list(range(_NCORES)), trace=trace)
    outs = []
    for i in range(_NCORES):
        o = (
            np.asarray(res.results[i]["out"])
            .reshape(_HO, _C, _WO)
            .transpose(1, 0, 2)
            .astype(np.float32)
        )
        tail = np.asarray(res.results[i]["tail"]).astype(np.float32)
        o[0:128, 250:_HO, :] = tail[:, 0]
        o[128:256, 250:_HO, :] = tail[:, 1]
        outs.append(o)
    return np.stack(outs, axis=0), res


def kernel(x, kernel):
    out, _ = _run(x, kernel, trace=False)
    return out



# revision 7
# speedup vs baseline: 1.0993x; 1.0993x over previous
"""Trainium2 Bass kernel for nn_Blur: depthwise 4x4 FIR conv, pad=2. v6.1.

v6 (277.7us): CB=16 channel batches, banded-matmul H-conv fused with
4-shift W-conv, f16 I/O, tail rows via DVE FMA from a strip buffer.
v6.1: f16 strip (-1.3MB DMA), xt0/xt1 loads issued before strip/wf to
shrink pipeline fill, tail FMAs paced 2/iter from iter 8 so the tail
store lands before the drain, NBX=6 prefetch depth.
"""

import numpy as np

_C, _H, _W = 256, 256, 256
_HO, _WO = 257, 257
_NCORES = 8
_TILES = [(0, 125, 0, 126), (125, 125, 123, 128)]
_XW = 262  # per-channel padded width: 2 zero | 256 data | 4 zero
_NMM = 258
_CB = 16  # channels per DMA batch; 2 psum sub-batches of 8
_NTAIL = 7
_FMA_START = 8  # 32 tail FMAs paced 2/iter starting at iteration 8


def _build_bands(kern):
    wf = np.ascontiguousarray(np.asarray(kern, np.float32)[::-1, ::-1])
    bands = np.zeros((128, 2, 4, 125), np.float32)
    for v, (hp0, Mv, hlo, Kv) in enumerate(_TILES):
        for j in range(4):
            for hr in range(Kv):
                h = hlo + hr
                for mr in range(Mv):
                    i = h - (hp0 + mr) + 2
                    if 0 <= i < 4:
                        bands[hr, v, j, mr] = wf[i, j]
    return bands.astype(np.float16), wf


_NC_CACHE = {}


def _build_nc():
    if "nc" in _NC_CACHE:
        return _NC_CACHE["nc"]
    import concourse.bacc as bacc
    import concourse.mybir as mybir
    import concourse.tile as tile

    f16 = mybir.dt.float16
    f32 = mybir.dt.float32

    nc = bacc.Bacc()
    x_d = nc.declare_dram_parameter("x", [_H, _C * _XW], f16, isOutput=False)
    b_d = nc.declare_dram_parameter("bands", [128, 2, 4, 125], f16, isOutput=False)
    s_d = nc.declare_dram_parameter("strip", [128, 2, 10, 260], f16, isOutput=False)
    w_d = nc.declare_dram_parameter("wfbc", [128, 16], f32, isOutput=False)
    o_d = nc.declare_dram_parameter("out", [_HO, _C * _WO], f16, isOutput=True)
    t_d = nc.declare_dram_parameter("tail", [128, 2, _NTAIL, _WO], f16, isOutput=True)

    NBX = 6
    NBO = 4
    NBP = 8
    PF = 3  # xt prefetch distance
    XTW = _CB * _XW  # 4192
    OSW = _CB * _WO  # 4112
    TAPS = [(i, j) for i in range(4) for j in range(4)]
    with tile.TileContext(nc) as tc:
        with (
            tc.tile_pool(name="sb", bufs=1) as pool,
            tc.tile_pool(name="ps", bufs=1, space="PSUM") as pp,
        ):
            band_sb = pool.tile([128, 2, 4, 125], f16, tag="bands")
            nc.sync.dma_start(out=band_sb[:], in_=b_d[:])
            strip_sb = pool.tile([128, 2, 10, 260], f16, tag="strip")
            wf_sb = pool.tile([128, 16], f32, tag="wf")

            xts = [
                pool.tile([128, XTW], f16, tag=f"xt{i}", name=f"xt{i}")
                for i in range(NBX)
            ]
            oss = [
                pool.tile([128, OSW], f16, tag=f"os{i}", name=f"os{i}")
                for i in range(NBO)
            ]
            pss = [
                pp.tile([128, 512], f32, tag=f"ps{i}", name=f"ps{i}")
                for i in range(NBP)
            ]
            accA = pool.tile([128, 2, _NTAIL, _WO], f32, tag="accA")
            accB = pool.tile([128, 2, _NTAIL, _WO], f32, tag="accB")
            acc16 = pool.tile([128, 2, _NTAIL, _WO], f16, tag="acc16")

            def emit_tail_fma(k):
                g, t = k % 2, k // 2
                i, j = TAPS[t]
                src = strip_sb[:, g, i : i + _NTAIL, j : j + _WO]
                sc = wf_sb[:, t : t + 1]
                if t == 0:
                    nc.vector.tensor_scalar_mul(accA[:, g], src, sc)
                else:
                    dst, prev = (accA, accB) if t % 2 == 0 else (accB, accA)
                    nc.vector.scalar_tensor_tensor(
                        out=dst[:, g],
                        in0=src,
                        scalar=sc,
                        in1=prev[:, g],
                        op0=mybir.AluOpType.mult,
                        op1=mybir.AluOpType.add,
                    )
                if t == 15:
                    nc.vector.tensor_copy(acc16[:, g], accB[:, g])
                    if g == 1:
                        nc.gpsimd.dma_start(out=t_d[:], in_=acc16[:])

            sched = [
                (c0, v) for c0 in range(0, _C, _CB) for v in range(len(_TILES))
            ]

            def load_xt(i):
                c0, v = sched[i]
                _, _, hlo, Kv = _TILES[v]
                nc.sync.dma_start(
                    out=xts[i % NBX][0:Kv, 0:XTW],
                    in_=x_d[hlo : hlo + Kv, c0 * _XW : c0 * _XW + XTW],
                )

            for i in range(PF):
                load_xt(i)
            nc.sync.dma_start(out=strip_sb[:], in_=s_d[:])
            nc.sync.dma_start(out=wf_sb[:], in_=w_d[:])

            for it, (c0, v) in enumerate(sched):
                hp0, Mv, hlo, Kv = _TILES[v]
                xt = xts[it % NBX]
                osb = oss[it % NBO]
                if it >= _FMA_START:
                    for k in (2 * (it - _FMA_START), 2 * (it - _FMA_START) + 1):
                        if 0 <= k < 32:
                            emit_tail_fma(k)
                if it + PF < len(sched):
                    load_xt(it + PF)
                for half in range(2):
                    for j in range(4):
                        for c8 in range(8):
                            cc = half * 8 + c8
                            nc.tensor.matmul(
                                pss[c8][0:Mv, 0:_NMM],
                                band_sb[0:Kv, v, j, 0:Mv],
                                xt[0:Kv, cc * _XW + j : cc * _XW + j + _NMM],
                                start=(j == 0),
                                stop=(j == 3),
                            )
                    for c8 in range(8):
                        cc = half * 8 + c8
                        ps = pss[c8]
                        if c8 < 3:
                            nc.vector.tensor_copy(
                                osb[0:Mv, cc * _WO : cc * _WO + _WO],
                                ps[0:Mv, 0:_WO],
                            )
                        else:
                            nc.scalar.copy(
                                osb[0:Mv, cc * _WO : cc * _WO + _WO],
                                ps[0:Mv, 0:_WO],
                            )
                nc.gpsimd.dma_start(
                    out=o_d[hp0 : hp0 + Mv, c0 * _WO : c0 * _WO + OSW],
                    in_=osb[0:Mv, 0:OSW],
                )
    nc.finalize()
    _NC_CACHE["nc"] = nc
    return nc


def _prep_core_inputs(x, bands16, wfbc, b):
    xb = x[b]  # [C, H, W] f32
    xT = np.zeros((_H, _C, _XW), np.float16)
    xT[:, :, 2:258] = xb.transpose(1, 0, 2).astype(np.float16, order="C")
    strip = np.zeros((128, 2, 10, 260), np.float16)
    bot = xb[:, 248:256, :].astype(np.float16)
    strip[:, 0, 0:8, 2:258] = bot[0:128]
    strip[:, 1, 0:8, 2:258] = bot[128:256]
    return {
        "x": xT.reshape(_H, _C * _XW),
        "bands": bands16,
        "strip": strip,
        "wfbc": wfbc,
    }


def _run(x, kern, trace=False):
    from concourse.bass_utils import run_bass_kernel_spmd

    x = np.asarray(x, dtype=np.float32)
    bands16, wf = _build_bands(kern)
    wfbc = np.ascontiguousarray(
        np.broadcast_to(wf.reshape(1, 16), (128, 16)).astype(np.float32)
    )
    nc = _build_nc()
    in_maps = [_prep_core_inputs(x, bands16, wfbc, b) for b in range(_NCORES)]
    res = run_bass_kernel_spmd(nc, in_maps, list(range(_NCORES)), trace=trace)
    outs = []
    for i in range(_NCORES):
        o = (
            np.asarray(res.results[i]["out"])
            .reshape(_HO, _C, _WO)
            .transpose(1, 0, 2)
            .astype(np.float32)
        )
        tail = np.asarray(res.results[i]["tail"]).astype(np.float32)
        o[0:128, 250:_HO, :] = tail[:, 0]
        o[128:256, 250:_HO, :] = tail[:, 1]
        outs.append(o)
    return np.stack(outs, axis=0), res


def kernel(x, kernel):
    out, _ = _run(x, kernel, trace=False)
    return out

